# revision 4
# baseline (speedup 1.0000x reference)
# Multi-head attention (B=2, S=2048, D=1024, H=16) on 8 Trainium2 NeuronCores.
#
# Sharding: tensor-parallel over heads (2 heads/core) for QKV + attention,
# chunked AllGather (one per batch element) of the normalized per-head
# context in bf16, and a feature-sharded output projection (each core
# computes 128 output channels for all positions).
#
# Schedule: one flat software pipeline over all qblocks of all reps.  At
# pipeline slot t the emitter interleaves, piecewise:
#   - attention for qblock t (8 score groups; score-matmuls -> exp on the
#     scalar engine -> ctx-accumulate matmuls, ctx lagging 2 groups),
#   - QKV projection pieces for qblock t+4 (whose batch's attention starts
#     at slot t+4), as PE filler between dependent score/ctx bursts,
#   - out-projection for qblock t-7 (whose AllGather was kicked at its
#     batch boundary), as more PE filler, with its gathered-context DMA
#     prefetched at slot top.
# This keeps the tensor engine continuously busy (full 2.4 GHz p-state;
# stalls drop it to 1.2 GHz) and hides the collective + exp latency.
#
# Precision: scores path is fp32r end-to-end (Q/K quantization feeds
# through exp as an absolute logit error, so bf16 there would cost ~1.5e-2
# L2).  P (post-exp), V, the gathered context and Wo are bf16: those enter
# linearly, so ~4e-3 element noise stays ~4e-3 in L2, well inside the
# 2e-2 gate, and it halves collective + SBUF traffic.  PSUM accumulation
# is fp32 everywhere.  V-transposes run on the PE (f32, sharing the lin
# psum tag); all 8 PSUM banks: scores 2x2, ctx 2x1, linears 2x1.
#
# Softmax: scores are O(1) here so exp() without max-subtraction is exact
# up to rounding.  V rows are pre-multiplied by the mask and the mask
# vector rides along as lhsT column 64, so the softmax denominator
# sum(P * mask) falls out of the same PE accumulation as the context.

import numpy as np

B, S, D, H, HD = 2, 2048, 1024, 16, 64
N_CORES = 8
BS = B * S            # 4096 total positions
DPC = D // N_CORES    # 128 channels per core (2 heads)
QB = 512              # query-block columns (one PSUM bank of fp32)
NBLK = BS // QB       # 8 query blocks
NKT = S // 128        # 16 key tiles per batch element
NE = D // 128         # 8 contraction chunks of the model dim
NG = 2                # key tiles per score group (psum [128, NG*QB])
NSG = NKT // NG       # 8 score groups per qblock
ALEAD = NBLK // B     # projection lead (slots): one batch of qblocks
_CACHE = {}


def _build_nc(nreps=1, no_collective=False, stages=3, ag_chunks=2, debug=False,
              tr_mode="pe", lag=2, ng=NG):
    import concourse.mybir as mybir
    import concourse.tile as tile
    from concourse import bacc
    from concourse.masks import make_identity

    F32 = mybir.dt.float32
    F32R = mybir.dt.float32r
    BF16 = mybir.dt.bfloat16
    EXP = mybir.ActivationFunctionType.Exp

    nc = bacc.Bacc(None, target_bir_lowering=False, num_devices=N_CORES)

    xT_d = nc.dram_tensor("xT", [D, BS], F32R, kind="ExternalInput")
    mask_d = nc.dram_tensor("maskf", [BS], F32, kind="ExternalInput")
    w_d = {}
    b_d = {}
    for nm in ("wq", "wk", "wv"):
        w_d[nm] = nc.dram_tensor(nm, [D, DPC], F32R, kind="ExternalInput")
    w_d["wo"] = nc.dram_tensor("wo", [D, DPC], BF16, kind="ExternalInput")
    for nm in ("bq", "bk", "bv", "bo"):
        b_d[nm] = nc.dram_tensor(nm, [DPC, 1], F32, kind="ExternalInput")
    out_d = nc.dram_tensor("outT", [DPC, BS], F32, kind="ExternalOutput")
    dbg = {}
    if debug:
        dbg["qT"] = nc.dram_tensor("qT_dbg", [128, BS], F32R, kind="ExternalOutput")
        dbg["kT"] = nc.dram_tensor("kT_dbg", [128, BS], F32R, kind="ExternalOutput")
        dbg["vT"] = nc.dram_tensor("vT_dbg", [128, BS], F32, kind="ExternalOutput")
        for h in range(2):
            dbg[f"vp{h}"] = nc.dram_tensor(
                f"vp{h}_dbg", [128, (BS // 128) * 66], BF16, kind="ExternalOutput"
            )
        dbg["cn"] = nc.dram_tensor("cn_dbg", [DPC, BS], BF16, kind="ExternalOutput")

    T = NBLK * nreps              # total qblock slots
    NCC = NBLK // ag_chunks       # qblocks per collective chunk
    cc_in = [
        nc.dram_tensor(f"cc_in{i}", [DPC, NCC * QB], BF16)
        for i in range(ag_chunks * nreps)
    ]
    cc_out = [
        nc.dram_tensor(f"cc_out{i}", [D, NCC * QB], BF16, addr_space="Shared")
        for i in range(ag_chunks * nreps)
    ]
    rgroup = [list(range(N_CORES))]
    nsg = NKT // ng               # score groups per qblock

    with tile.TileContext(nc) as tc:
        with (
            tc.tile_pool(name="persist", bufs=1) as pp,
            tc.tile_pool(name="xp", bufs=2) as xp,
            tc.tile_pool(name="ptp", bufs=8) as ptp,
            tc.tile_pool(name="vtpp", bufs=4) as vtpp,
            tc.tile_pool(name="smal", bufs=4) as smal,
            tc.tile_pool(name="cnp", bufs=3) as cnp,
            tc.tile_pool(name="cgp", bufs=2) as cgp,
            tc.tile_pool(name="ps_lin", bufs=2, space="PSUM") as ps_lin,
            tc.tile_pool(name="ps_s", bufs=(2 if ng == 2 else 4), space="PSUM") as ps_s,
            tc.tile_pool(name="ps_ctx", bufs=2, space="PSUM") as ps_ctx,
        ):
            # ---------------- persistent state ----------------------------
            w_sb = {}
            for nm in ("wq", "wk", "wv", "wo"):
                dt = BF16 if nm == "wo" else F32R
                w_sb[nm] = pp.tile([128, D], dt, name=f"{nm}_sb")
                nc.sync.dma_start(
                    w_sb[nm][:].rearrange("p (c d) -> p c d", d=DPC),
                    w_d[nm][:].rearrange("(c p) d -> p c d", p=128),
                )
            b_sb = {}
            for nm in ("bq", "bk", "bv", "bo"):
                b_sb[nm] = pp.tile([DPC, 1], F32, name=f"{nm}_sb")
                nc.sync.dma_start(b_sb[nm][:], b_d[nm][:])
            maskt = pp.tile([128, BS // 128], F32, name="maskt")
            nc.sync.dma_start(maskt[:], mask_d[:].rearrange("(t p) -> p t", p=128))

            ident = pp.tile([128, 128], F32, name="ident")
            make_identity(nc, ident[:])
            qT = pp.tile([128, BS], F32R, name="qT")
            kT = pp.tile([128, BS], F32R, name="kT")
            vT = pp.tile([128, BS], BF16 if tr_mode == "xbar" else F32, name="vT")
            # V' per head: [128, 65] per key tile; col 64 is the mask
            # column, written once here (the mask is constant across reps).
            vp = [
                pp.tile([128, (BS // 128) * 66], BF16, name=f"vp{h}")
                for h in range(2)
            ]
            for t in range(BS // 128):
                for h in range(2):
                    nc.vector.tensor_copy(
                        vp[h][:, 66 * t + 64 : 66 * t + 65], maskt[:, t : t + 1]
                    )

            # ---------------- pipeline units -------------------------------
            def a_xdma(q):
                j = q % NBLK
                cols = slice(QB * j, QB * (j + 1))
                xct = xp.tile([128, NE, QB], F32R, name="xct", tag="xc")
                for e in range(0, NE, 4):
                    nc.sync.dma_start(
                        xct[:, e : e + 4, :],
                        xT_d[128 * e : 128 * (e + 4), cols].rearrange(
                            "(c p) q -> p c q", p=128
                        ),
                    )
                return xct

            def a_proj(q, xct, which):
                # one projection (q/k/v) of qblock q: 8 matmuls + bias add
                j = q % NBLK
                cols = slice(QB * j, QB * (j + 1))
                nm = ("wq", "wk", "wv")[which]
                ps = ps_lin.tile([128, QB], F32, name=f"ps_{nm}", tag="lin")
                for e in range(NE):
                    nc.tensor.matmul(
                        ps[:],
                        w_sb[nm][:, 128 * e : 128 * (e + 1)],
                        xct[:, e, :],
                        start=(e == 0),
                        stop=(e == NE - 1),
                    )
                dst = (qT, kT, vT)[which]
                bnm = ("bq", "bk", "bv")[which]
                nc.vector.tensor_scalar_add(
                    dst[:, cols], ps[:], b_sb[bnm][:, 0:1]
                )

            def a_tr(q, half):
                # transpose + mask-fold half this qblock's V tiles.
                # tr_mode "pe": PE transpose into the shared lin psum tag +
                # DVE fold reading f32 psum (the fold must be mixed-dtype:
                # an all-bf16 DVE tensor_scalar hits the 2x-packed DVE
                # mode, which corrupts every even partition).  Split in
                # halves so the second pair's psum-slot WAR (on the first
                # pair's DVE folds) resolves before the PE reaches it.
                # tr_mode "xbar": DMA XBAR transpose (16-bit only) into
                # SBUF + ACT-engine Copy-with-scale fold (no PE cycles,
                # and "copy" shares exp's activation table set).
                j = q % NBLK
                COPY = mybir.ActivationFunctionType.Copy
                for t in range(4 * j + 2 * half, 4 * j + 2 * half + 2):
                    if tr_mode == "xbar":
                        vtp = vtpp.tile([128, 128], BF16, name="vtp", tag="vtp")
                        nc.sync.dma_start(
                            vtp[:], vT[:, 128 * t : 128 * (t + 1)], transpose=True
                        )
                        for h in range(2):
                            nc.scalar.activation(
                                vp[h][:, 66 * t : 66 * t + 64],
                                vtp[:, 64 * h : 64 * (h + 1)],
                                COPY,
                                scale=maskt[:, t : t + 1],
                            )
                        continue
                    vtp = ps_lin.tile([128, 128], F32, name="vtp", tag="lin")
                    nc.tensor.transpose(
                        vtp[:], vT[:, 128 * t : 128 * (t + 1)], ident[:]
                    )
                    for h in range(2):
                        nc.vector.tensor_scalar_mul(
                            vp[h][:, 66 * t : 66 * t + 64],
                            vtp[:, 64 * h : 64 * (h + 1)],
                            maskt[:, t : t + 1],
                        )

            def b_sg(q, G):
                # score matmuls for group G (2 key tiles x 2 heads) + exp
                j = q % NBLK
                b = j // (NBLK // B)
                qcols = slice(QB * j, QB * (j + 1))
                pts = []
                for h in range(2):
                    sp = ps_s.tile([128, ng * QB], F32, name=f"s{h}", tag="s")
                    hrow = slice(64 * h, 64 * (h + 1))
                    for u in range(ng):
                        kt = ng * G + u
                        kcols = slice(S * b + 128 * kt, S * b + 128 * (kt + 1))
                        nc.tensor.matmul(
                            sp[:, QB * u : QB * (u + 1)],
                            kT[hrow, kcols],
                            qT[hrow, qcols],
                            start=True,
                            stop=True,
                            tile_position=(64 * h, 0),
                        )
                    pt = ptp.tile([128, ng * QB], BF16, name="pt", tag="pt")
                    nc.scalar.activation(pt[:], sp[:], EXP, scale=0.125)
                    pts.append(pt)
                return pts

            def b_cg(q, G, pts, ctx):
                j = q % NBLK
                b = j // (NBLK // B)
                for h in range(2):
                    for u in range(ng):
                        kt = ng * G + u
                        vtile = NKT * b + kt
                        nc.tensor.matmul(
                            ctx[h][:],
                            vp[h][:, 66 * vtile : 66 * vtile + 65],
                            pts[h][:, QB * u : QB * (u + 1)],
                            start=(G == 0 and u == 0),
                            stop=(G == nsg - 1 and u == ng - 1),
                        )

            def b_norm(q, ctx, cn, h):
                # normalize head h's context rows by the accumulated denom
                den = smal.tile([1, QB], F32, name="den", tag="den")
                nc.vector.tensor_copy(den[:], ctx[h][64:65, :])
                recip = smal.tile([1, QB], F32, name="recip", tag="recip")
                nc.vector.reciprocal_approx_fast(recip[:], den[:])
                rb = smal.tile([64, QB], F32, name="rb", tag="rb")
                nc.gpsimd.partition_broadcast(rb[:], recip[:])
                nc.vector.tensor_mul(
                    cn[64 * h : 64 * (h + 1), :], ctx[h][0:64, :], rb[:]
                )

            def b_ccin(q, cn):
                chunk, pos = divmod(q, NCC)
                nc.sync.dma_start(cn_cols(chunk, pos), cn[:])

            def cn_cols(chunk, pos):
                return cc_in[chunk][:, QB * pos : QB * (pos + 1)]

            def b_ag(q):
                if (q + 1) % NCC:
                    return
                chunk = q // NCC
                if no_collective:
                    return  # timing-only variant: out-proj reads stale cc_out
                if True:
                    nc.gpsimd.collective_compute(
                        "AllGather",
                        mybir.AluOpType.bypass,
                        replica_groups=rgroup,
                        ins=[cc_in[chunk][:].opt()],
                        outs=[cc_out[chunk][:].opt()],
                    )

            def ob_dma(q):
                chunk, pos = divmod(q, NCC)
                hcols = slice(QB * pos, QB * (pos + 1))
                cg = cgp.tile([128, NE, QB], BF16, name="cg", tag="cg")
                for c in range(NE):
                    nc.sync.dma_start(
                        cg[:, c, :], cc_out[chunk][128 * c : 128 * (c + 1), hcols]
                    )
                return cg

            def ob_compute(qo, cg):
                # out-projection matmuls for qblock qo (one whole unit: the
                # lin-tag psum slot must not survive a slot boundary, or
                # the next a_proj allocation would rotate into it mid-use)
                o_ps = ps_lin.tile([128, QB], F32, name="o_ps", tag="lin")
                for c in range(NE):
                    nc.tensor.matmul(
                        o_ps[:],
                        w_sb["wo"][:, 128 * c : 128 * (c + 1)],
                        cg[:, c, :],
                        start=(c == 0),
                        stop=(c == NE - 1),
                    )
                j = qo % NBLK
                oc = smal.tile([128, QB], F32, name="oc", tag="oc")
                nc.vector.tensor_scalar_add(oc[:], o_ps[:], b_sb["bo"][:, 0:1])
                nc.sync.dma_start(out_d[:, QB * j : QB * (j + 1)], oc[:])

            # ---------------- emission -------------------------------------
            OBLAG = max(ALEAD + 3, NCC + 3)  # out-proj trails its AllGather

            def emit_a(q):
                xct = a_xdma(q)
                for w in range(3):
                    a_proj(q, xct, w)
                a_tr(q, 0)
                a_tr(q, 1)

            if stages < 2:
                for q in range(T):
                    emit_a(q)
            else:
                for q in range(min(ALEAD, T)):
                    emit_a(q)
                xct_pending = {}
                if ALEAD < T:
                    xct_pending[ALEAD] = a_xdma(ALEAD)

                for t in range(T):
                    # prefetches: gathered context for this slot's
                    # out-projection first (it feeds the PE sooner), then
                    # the x tile for slot t+1's projections
                    qo = t - OBLAG
                    ob = stages >= 3 and 0 <= qo < T
                    cg = ob_dma(qo) if ob else None
                    if t + ALEAD + 1 < T:
                        xct_pending[t + ALEAD + 1] = a_xdma(t + ALEAD + 1)
                    qa = t + ALEAD
                    xct = xct_pending.pop(qa, None)
                    ctx = [
                        ps_ctx.tile([65, QB], F32, name=f"ctx{h}", tag="ctx")
                        for h in range(2)
                    ]
                    cn = cnp.tile([128, QB], BF16, name="cn", tag="cn")
                    # score groups run `lag` ahead of ctx accumulation so
                    # the exp latency and the previous slot's norm chain
                    # are both off the PE critical path; filler pieces
                    # (proj / transpose / out-proj) sit between dependent
                    # bursts.
                    fillers = []
                    if xct is not None:
                        fillers += [
                            lambda w=w: a_proj(qa, xct, w) for w in range(3)
                        ]
                        fillers.append(lambda: a_tr(qa, 0))
                    if ob:
                        fillers.append(lambda: ob_compute(qo, cg))
                    if xct is not None:
                        fillers.append(lambda: a_tr(qa, 1))
                    pts = {}
                    for G in range(nsg):
                        pts[G] = b_sg(t, G)
                        if G - lag >= 0:
                            b_cg(t, G - lag, pts.pop(G - lag), ctx)
                        if fillers:
                            fillers.pop(0)()
                    for G in range(nsg - lag, nsg):
                        b_cg(t, G, pts.pop(G), ctx)
                    while fillers:
                        fillers.pop(0)()
                    for h in range(2):
                        b_norm(t, ctx, cn, h)
                    if debug:
                        j = t % NBLK
                        nc.sync.dma_start(
                            dbg["cn"][:, QB * j : QB * (j + 1)], cn[:]
                        )
                    if stages >= 3:
                        b_ccin(t, cn)
                        b_ag(t)
                # drain the out-projection pipeline
                if stages >= 3:
                    for t in range(T, T + OBLAG):
                        qo = t - OBLAG
                        if 0 <= qo < T:
                            ob_compute(qo, ob_dma(qo))
            if debug:
                nc.sync.dma_start(dbg["qT"][:], qT[:])
                nc.sync.dma_start(dbg["kT"][:], kT[:])
                nc.sync.dma_start(dbg["vT"][:], vT[:])
                for h in range(2):
                    nc.sync.dma_start(dbg[f"vp{h}"][:], vp[h][:])

    nc.compile()
    return nc


def _get_nc(nreps=1, no_collective=False):
    key = (nreps, no_collective)
    if key not in _CACHE:
        _CACHE[key] = _build_nc(nreps, no_collective)
    return _CACHE[key]


def _make_in_maps(x, mask, Wq, bq, Wk, bk, Wv, bv, Wo, bo):
    import ml_dtypes

    f32 = np.float32
    bf16 = ml_dtypes.bfloat16
    x = np.asarray(x, f32)
    xT = np.ascontiguousarray(x.reshape(BS, D).T)
    maskf = np.asarray(mask).astype(f32).reshape(BS)
    Ws = {"wq": np.asarray(Wq, f32), "wk": np.asarray(Wk, f32), "wv": np.asarray(Wv, f32), "wo": np.asarray(Wo, f32)}
    bs = {"bq": np.asarray(bq, f32), "bk": np.asarray(bk, f32), "bv": np.asarray(bv, f32), "bo": np.asarray(bo, f32)}
    in_maps = []
    for r in range(N_CORES):
        rows = slice(DPC * r, DPC * (r + 1))
        m = {"xT": xT, "maskf": maskf}
        for nm, W in Ws.items():
            wr = np.ascontiguousarray(W[rows].T)
            m[nm] = wr.astype(bf16) if nm == "wo" else wr
        for nm, b in bs.items():
            m[nm] = np.ascontiguousarray(b[rows].reshape(DPC, 1))
        in_maps.append(m)
    return in_maps


def kernel(x, mask, Wq, bq, Wk, bk, Wv, bv, Wo, bo):
    from concourse import bass_utils

    nc = _get_nc()
    in_maps = _make_in_maps(x, mask, Wq, bq, Wk, bk, Wv, bv, Wo, bo)
    try:
        res = bass_utils.run_bass_kernel_spmd(
            nc, in_maps, core_ids=list(range(N_CORES))
        )
    except Exception:
        # one retry: a previously-crashed run can leave a core wedged and
        # fail the first execution afterwards
        res = bass_utils.run_bass_kernel_spmd(
            nc, in_maps, core_ids=list(range(N_CORES))
        )
    outT = np.concatenate([res.results[r]["outT"] for r in range(N_CORES)], axis=0)
    return np.ascontiguousarray(outT.T).reshape(B, S, D).astype(np.float32)
